# revision 95
# baseline (speedup 1.0000x reference)
"""GQA (32 q heads / 8 kv heads, RoPE, causal) Trainium2 Bass kernel.

Sharding: tensor-parallel over kv heads -- core c owns kv head c and q heads
4c..4c+3 for both batches. Each core computes a partial o-projection
(its 256 attn channels x Wo columns) in fp16 and the host sums the 8 partials.

Device-side structure (per core, per batch):
  * QKV projection in fp16: stationary x chunks [d,128 tok], moving fused
    W [d, 384] = [4 q heads | k | v]; psum evicted to fp16 by DVE.
  * RoPE on GPSIMD in token-partition layout (keeps DVE clear for the
    attention stream); head dims are host-permuted to [32 evens | 32 odds]
    so the pair slices are stride-1.
  * Q/K transposed to [dh, tok] via DMA-XBAR transposes (no PE involvement);
    K replicated to partitions 64:127 with one SBUF-to-SBUF DMA per batch so
    odd heads can use matching base partitions.
  * Scores computed [keys, q] per 128-key chunk with the moving operand
    clipped to the causal range (exact-causal FLOPs/exp); exp on ACT
    (scale=1/8, no max needed); diagonal triangle masked by DVE fp16 mul.
  * attn.V restructured: out [q, dh+1] psum (65 rows per 128-key chunk, half
    the PE cost of the [dh, q] form); ones column of V gives the softmax
    denominator; normalization = DVE reciprocal + per-partition scalar mul.
    Each of the 4 q-subchunk accumulators owns a full psum bank (hardware
    accumulation groups are per-bank and must not interleave within one).
  * Normalized attn tiles [q, dh] are pair-packed and DMA-XBAR-transposed
    into the o-projection stationary layout [channels, tok].
  * o-projection psum evicted to fp16 by DVE/ACT (GPSIMD cannot read PSUM);
    o written back fp16 by GPSIMD-issued SWDGE DMAs (off the shared HWDGE).
  * Cross-phase software pipelining: attention starts after 4 QKV tiles;
    remaining QKV(b0) tiles, QKV(b1), and o-proj(b0) PE work drain as
    background units between score pieces of the ACT-paced attention phases.
"""

import numpy as np
from collections import deque
from contextlib import ExitStack

import concourse.bass as bass
from concourse import bacc
import concourse.mybir as mybir
import concourse.tile as tile
from concourse.bass_utils import run_bass_kernel_spmd

B, S, D = 2, 2048, 2048
DH = 64            # head dim
G = 4              # q heads per core (= per kv head)
NCORES = 8
TT = 512           # attention i-tile
NTT = S // TT      # 4
KC = D // 128      # 16 contraction chunks
NJC = S // 128     # 16 token/key chunks of 128
F32 = mybir.dt.float32
F16 = mybir.dt.float16
ROPE_BASE = 10000.0

_cached = {}


def build_nc():
    nc = bacc.Bacc("TRN2", target_bir_lowering=False, debug=False)
    xt = nc.declare_dram_parameter("xt", [B, D, S], F16, isOutput=False)
    wall = nc.declare_dram_parameter("wall", [D, 384], F16, isOutput=False)
    wot = nc.declare_dram_parameter("wot", [256, D], F16, isOutput=False)
    cosr = nc.declare_dram_parameter("cosr", [S, 32], F16, isOutput=False)
    sinr = nc.declare_dram_parameter("sinr", [S, 32], F16, isOutput=False)
    cmask = nc.declare_dram_parameter("cmask", [4, 128, TT], F16, isOutput=False)
    o = nc.declare_dram_parameter("o", [B, S, D], F16, isOutput=True)

    EXP = mybir.ActivationFunctionType.Exp

    with tile.TileContext(nc) as tc, ExitStack() as ctx:
        wpool = ctx.enter_context(tc.tile_pool(name="weights", bufs=1))
        per_b = ctx.enter_context(tc.tile_pool(name="per_b", bufs=1))
        xpool = ctx.enter_context(tc.tile_pool(name="xstream", bufs=4))
        qkvpool = ctx.enter_context(tc.tile_pool(name="qkv", bufs=4))
        epool = ctx.enter_context(tc.tile_pool(name="exp", bufs=8))
        rpool = ctx.enter_context(tc.tile_pool(name="rope", bufs=3))
        opool = ctx.enter_context(tc.tile_pool(name="out", bufs=8))
        spool = ctx.enter_context(tc.tile_pool(name="small", bufs=4))
        # psum budget (8 banks): pgen 2 (shared by qkv-proj and o-proj),
        # scores 2, av accumulators 4.
        pp_gen = ctx.enter_context(tc.tile_pool(name="pgen", bufs=2, space="PSUM"))
        pp_att = ctx.enter_context(tc.tile_pool(name="patt", bufs=2, space="PSUM"))
        pp_av = ctx.enter_context(tc.tile_pool(name="pav", bufs=1, space="PSUM"))

        # ---- persistent weights/tables ----
        wall_sb = wpool.tile([128, KC, 384], F16, tag="wall")
        wot_sb = wpool.tile([128, 2, D], F16, tag="wot")
        cos_sb = wpool.tile([128, NJC, 32], F16, tag="cos")
        sin_sb = wpool.tile([128, NJC, 32], F16, tag="sin")
        mask_sb = wpool.tile([128, 1, 128], F16, tag="mask")

        def load_wall_head():
            nc.sync.dma_start(wall_sb[:, 0:4, :],
                              wall.rearrange("(kc p) n -> p kc n", p=128)[:, 0:4, :])

        def load_weights():
            nc.sync.dma_start(wall_sb[:, 4:KC, :],
                              wall.rearrange("(kc p) n -> p kc n", p=128)[:, 4:KC, :])
            nc.sync.dma_start(cos_sb[:],
                              cosr.rearrange("(j p) n -> p j n", p=128))
            nc.sync.dma_start(sin_sb[:],
                              sinr.rearrange("(j p) n -> p j n", p=128))

        def load_weights_late():
            # needed from the first attention diagonal onward
            nc.sync.dma_start(mask_sb[:, 0, 0:128], cmask[0][:, 0:128])
            # warm the Exp activation table off the critical path
            warm = spool.tile([128, 1], F32, tag="warm")
            nc.vector.memset(warm[:], 0.0)
            nc.scalar.activation(warm[:], warm[:], EXP, scale=1.0)

        def load_wot():
            # needed only by the o-projection, much later
            nc.sync.dma_start(wot_sb[:],
                              wot.rearrange("(cc p) n -> p cc n", p=128))

        # per-batch persistent tiles
        qt = {}     # [128, 2, S]: pair p holds heads 2p (part 0:64), 2p+1 (64:128)
        ktv = {}    # [128, S]: rows 0:64 = K^T; rows 64:128 = V^T (unused)
        ktv2 = {}   # [128, S]: rows 64:128 = K^T copy (for odd heads)
        vsb = {}    # [128, NJC, 65]: V natural [tok, dh] + ones column
        at = {}     # [128, 2, S]: o-proj stationary (channels x tokens)
        nrm = {}    # [128, 2, NJC, 128]: normalized attn [q, dh] pair-packed
        for b in range(B):
            qt[b] = per_b.tile([128, 2, S], F16, tag=f"qt{b}", name=f"qt{b}")
            ktv[b] = per_b.tile([128, S], F16, tag=f"ktv{b}", name=f"ktv{b}")
            ktv2[b] = per_b.tile([128, S], F16, tag=f"ktv2{b}", name=f"ktv2{b}")
            vsb[b] = per_b.tile([128, NJC, 65], F16, tag=f"vsb{b}", name=f"vsb{b}")
            at[b] = per_b.tile([128, 2, S], F16, tag=f"at{b}", name=f"at{b}")
            nrm[b] = per_b.tile([128, 2, NJC, 128], F16, tag=f"nrm{b}",
                                name=f"nrm{b}")
            nc.vector.memset(vsb[b][:, :, 64:65], 1.0)

        # ---------- QKV projection + rope (emitted as interleavable units) ----
        def qkv_units(b):
            """Closures, each emitting a quarter-tile of projection work
            (4 PE matmuls); the last quarter adds DVE evict/rope + SP
            transposes. Fine granularity lets attention interleave them."""
            units = []
            xts = {}
            pqs = {}

            def load_x(tg):
                def emit():
                    xts[tg] = xpool.tile([128, KC, TT], F16, tag="xt",
                                         name=f"xt_{b}_{tg}")
                    nc.sync.dma_start(
                        xts[tg][:],
                        xt[b].rearrange("(kc p) s -> p kc s", p=128)
                        [:, :, tg * TT:(tg + 1) * TT])
                return emit

            def make_unit(tt, quarter):
                def emit():
                    tg = tt // 4
                    xtile = xts[tg]
                    s0 = (tt % 4) * 128
                    if quarter == 0:
                        pqs[tt] = pp_gen.tile([128, TT], F32, tag="pg",
                                              name=f"pq_{b}_{tt}")
                    pq = pqs[tt]
                    for k in range(quarter * 4, quarter * 4 + 4):
                        nc.tensor.matmul(pq[:, 0:384], xtile[:, k, s0:s0 + 128],
                                         wall_sb[:, k, :],
                                         start=(k == 0), stop=(k == KC - 1))
                    if quarter < 3:
                        return
                    del pqs[tt]
                    qkv = qkvpool.tile([128, 384], F16, tag="qkv")
                    nc.vector.tensor_copy(qkv[:], pq[:, 0:384])
                    # rope on q+k (cols 0:320); host permuted each head's dims
                    # to [32 evens | 32 odds] so these slices are stride-1
                    pear = qkv[:, 0:320].rearrange("p (h half i) -> p h half i",
                                                   half=2, i=32)
                    ev, od = pear[:, :, 0, :], pear[:, :, 1, :]
                    # one 32-wide table broadcast (stride-0) across 5 heads
                    cs = cos_sb[:, tt:tt + 1, :].broadcast_to([128, 5, 32])
                    sn = sin_sb[:, tt:tt + 1, :].broadcast_to([128, 5, 32])
                    ec = rpool.tile([128, 5, 32], F16, tag="ec")
                    es = rpool.tile([128, 5, 32], F16, tag="es")
                    oc = rpool.tile([128, 5, 32], F16, tag="oc")
                    os_ = rpool.tile([128, 5, 32], F16, tag="os")
                    nc.gpsimd.tensor_mul(ec[:], ev, cs)
                    nc.gpsimd.tensor_mul(es[:], ev, sn)
                    nc.gpsimd.tensor_mul(oc[:], od, cs)
                    nc.gpsimd.tensor_mul(os_[:], od, sn)
                    nc.gpsimd.tensor_sub(ev, ec[:], os_[:])
                    nc.gpsimd.tensor_add(od, es[:], oc[:])
                    nc.gpsimd.tensor_copy(vsb[b][:, tt, 0:64], qkv[:, 320:384])
                    tsl = slice(tt * 128, (tt + 1) * 128)
                    nc.sync.dma_start_transpose(qt[b][:, 0, tsl], qkv[:, 0:128])
                    nc.sync.dma_start_transpose(qt[b][:, 1, tsl], qkv[:, 128:256])
                    nc.sync.dma_start_transpose(ktv[b][:, tsl], qkv[:, 256:384])
                return emit

            for tt in range(NJC):
                if tt % 4 == 0:
                    units.append(load_x(tt // 4))
                for quarter in range(4):
                    units.append(make_unit(tt, quarter))
            return units

        def emit_ktv2(b):
            # K^T replicated to partitions 64:127 (DMA moves across partitions)
            nc.sync.dma_start(ktv2[b][64:128, :], ktv[b][0:64, :])

        # ---------- attention (q-outer, exact-causal via clipped moving) ----
        # Per (g, 512-query tile): key chunks jc ascending; the moving operand
        # is clipped to the causal range q >= 128*jc, so score/exp work is the
        # exact causal set. The 4 q-subchunk accumulators [q, dh+1] each own a
        # full psum bank -- hardware accumulation groups are per-bank, so
        # groups may interleave across banks but never within one.
        def emit_attn(b, bg, front=0):
            """bg: deque of PE-work closures interleaved between score pieces.
            The first `front` units drain at 4/piece (they gate upcoming
            attention tiles); the rest spread uniformly."""
            n_pieces = G * sum(4 * it + 4 for it in range(NTT))
            quota = [0] * n_pieces
            nfront = min(front, len(bg))
            pf = (nfront + 3) // 4
            for i in range(pf):
                quota[i] = min(4, nfront - 4 * i)
            rest = len(bg) - nfront
            if rest > 0:
                for i, extra in enumerate(np.diff(np.round(
                        np.linspace(0, rest, n_pieces - pf + 1)).astype(int))):
                    quota[pf + i] += int(extra)
            piece_idx = 0
            for g in range(G):
                base, pair, cc = (g % 2) * 64, g // 2, g // 2
                kst = ktv[b] if g % 2 == 0 else ktv2[b]
                for it in range(NTT):
                    avs = [pp_av.tile([128, TT], F32, tag=f"av{sub}",
                                      name=f"av_{b}_{g}_{it}_{sub}")
                           for sub in range(4)]
                    njc = 4 * it + 4
                    pending = []

                    def norm_one(qc, sub, avs=avs):
                        rcp = spool.tile([128, 1], F32, tag="rcp")
                        nc.vector.reciprocal(rcp[:], avs[sub][:, 64:65])
                        nc.vector.tensor_scalar_mul(
                            nrm[b][:, cc, qc, base:base + 64],
                            avs[sub][:, 0:64], rcp[:])

                    def flush_one(pending=pending, avs=avs, it=it):
                        jd, q0d, wd, ed = pending.pop(0)
                        for i in range(wd // 128):
                            qc = q0d // 128 + i
                            sub = qc - 4 * it
                            nc.tensor.matmul(
                                avs[sub][:, 0:65],
                                ed[:, i * 128:(i + 1) * 128],
                                vsb[b][:, jd, :],
                                start=(jd == 0), stop=(jd == qc))
                            if jd == qc:
                                norm_one(qc, sub)

                    for jc in range(njc):
                        jsl = slice(jc * 128, (jc + 1) * 128)
                        q0 = max(jc * 128, it * TT)
                        w = (it + 1) * TT - q0
                        psc = pp_att.tile([128, TT], F32, tag="sc")
                        nc.tensor.matmul(
                            psc[:, 0:w], kst[base:base + 64, jsl],
                            qt[b][base:base + 64, pair, q0:q0 + w],
                            start=True, stop=True)
                        esb = epool.tile([128, TT], F16, tag="exp")
                        nc.scalar.activation(esb[:, 0:w], psc[:, 0:w], EXP,
                                             scale=0.125)
                        if q0 == jc * 128:  # diagonal block: triangular mask
                            nc.vector.tensor_mul(esb[:, 0:128], esb[:, 0:128],
                                                 mask_sb[:, 0, 0:128])
                        pending.append(((0, jc, q0, w),))
                        pending[-1] = (jc, q0, w, esb)
                        if len(pending) > 3:
                            flush_one()
                        for _ in range(quota[piece_idx]):
                            if bg:
                                bg.popleft()()
                        piece_idx += 1
                    while pending:
                        flush_one()
                if g % 2 == 1:  # pair done: transpose into o-proj layout
                    for qc in range(NJC):
                        nc.sync.dma_start_transpose(
                            at[b][:, cc, qc * 128:(qc + 1) * 128],
                            nrm[b][:, cc, qc, :])
            while bg:
                bg.popleft()()

        # ---------- o projection (partial over this core's 256 channels) ----
        def oproj_units(b, evict_engines, pools):
            """evict_engines / pools: rotations for the psum->SBUF eviction
            engine and the psum pool (spread so neither serializes)."""
            units = []
            obs = {}

            def make_unit(tt, nt):
                def emit():
                    tsl = slice(tt * 128, (tt + 1) * 128)
                    if nt == 0:
                        obs[tt] = opool.tile([128, 4, TT], F16, tag="osb",
                                             name=f"osb_{b}_{tt}")
                    ob = obs[tt]
                    nsl = slice(nt * TT, (nt + 1) * TT)
                    pool = pools[(tt * 4 + nt) % len(pools)]
                    po = pool.tile([128, TT], F32,
                                   tag="pg" if pool is pp_gen else "sc",
                                   name=f"po_{b}_{tt}_{nt}")
                    nc.tensor.matmul(po[:], at[b][:, 0, tsl],
                                     wot_sb[:, 0, nsl], start=True, stop=False)
                    nc.tensor.matmul(po[:], at[b][:, 1, tsl],
                                     wot_sb[:, 1, nsl], start=False, stop=True)
                    eng = evict_engines[(tt * 4 + nt) % len(evict_engines)]
                    if eng is nc.scalar:
                        eng.copy(ob[:, nt, :], po[:])
                    else:
                        eng.tensor_copy(ob[:, nt, :], po[:])
                    if nt == 3:
                        del obs[tt]
                        nc.gpsimd.dma_start(o[b, tsl, :], ob[:])
                return emit

            for tt in range(NJC):
                for nt in range(4):
                    units.append(make_unit(tt, nt))
            return units

        # ---------- schedule ----------
        u0 = qkv_units(0)
        load_wall_head()  # first 4 contraction chunks of W: unblocks matmuls
        u0[0]()           # first x tile load
        load_weights()
        for u in u0[1:17]:  # tiles 0-3: enough to start attention it-tile 0
            u()
        load_weights_late()
        # remaining qkv(0) tiles drain front-loaded into attn(0) (they gate
        # its later it-tiles), then qkv(1) spreads across the rest
        u1 = qkv_units(1)
        ncut = 40  # last 4 tiles of qkv(1): only needed by attn(1) it-tile 3
        bg0 = deque(list(u0[17:]) + [lambda: emit_ktv2(0), load_wot]
                    + u1[:-ncut])
        emit_attn(0, bg0, front=len(u0) - 17 + 2)
        # during attn(1): evict on DVE (GPSIMD cannot read PSUM; ACT is
        # exp-bound) and stay off the attention psum pools
        bg1 = deque(u1[-ncut:] + [lambda: emit_ktv2(1)]
                    + oproj_units(0, [nc.vector], [pp_gen]))
        emit_attn(1, bg1, front=ncut + 1)
        # tail: every non-PE engine and the score psum pool are idle
        for u in oproj_units(1, [nc.vector, nc.scalar], [pp_gen, pp_att]):
            u()
    nc.compile()
    return nc


def host_inputs(x, Wq, Wk, Wv, Wo):
    """Per-core input maps. Q/K weight rows permuted per head to
    [32 even dims | 32 odd dims] so device rope slices are stride-1."""
    xtp = np.ascontiguousarray(
        np.transpose(np.asarray(x, np.float32), (0, 2, 1))).astype(np.float16)
    inv = ROPE_BASE ** (-np.arange(0, DH, 2, dtype=np.float64) / DH)
    th = np.arange(S, dtype=np.float64)[:, None] * inv[None, :]  # (S, 32)
    cosr = np.cos(th).astype(np.float16)  # (S, 32)
    sinr = np.sin(th).astype(np.float16)
    p = np.arange(128)[:, None]
    f = np.arange(TT)[None, :]
    cmask = np.stack([(p + m * 128 <= f).astype(np.float16) for m in range(4)])
    perm = np.concatenate([np.arange(0, DH, 2), np.arange(1, DH, 2)])
    in_maps = []
    for c in range(NCORES):
        qrows = [Wq[(4 * c + h) * DH:(4 * c + h + 1) * DH][perm] for h in range(G)]
        krows = Wk[DH * c:DH * (c + 1)][perm]
        vrows = Wv[DH * c:DH * (c + 1)]
        wall = np.concatenate(qrows + [krows, vrows], axis=0)     # (384, D)
        wall = np.ascontiguousarray(wall.T.astype(np.float16))    # (D, 384)
        wot = np.ascontiguousarray(Wo[:, 256 * c:256 * (c + 1)].T
                                   .astype(np.float16))           # (256, D)
        in_maps.append(dict(xt=xtp, wall=wall, wot=wot, cosr=cosr, sinr=sinr,
                            cmask=cmask))
    return in_maps


def kernel(**inputs):
    x = np.asarray(inputs["x"], dtype=np.float32)
    Wq = np.asarray(inputs["Wq"], dtype=np.float32)
    Wk = np.asarray(inputs["Wk"], dtype=np.float32)
    Wv = np.asarray(inputs["Wv"], dtype=np.float32)
    Wo = np.asarray(inputs["Wo"], dtype=np.float32)
    in_maps = host_inputs(x, Wq, Wk, Wv, Wo)
    if "nc" not in _cached:
        _cached["nc"] = build_nc()
    res = run_bass_kernel_spmd(_cached["nc"], in_maps, list(range(NCORES)))
    out = np.zeros((B, S, D), np.float64)
    for r in res.results:
        out += np.asarray(r["o"], np.float64)
    return out.astype(np.float32)


# revision 97
# speedup vs baseline: 1.0014x; 1.0014x over previous
"""GQA (32 q heads / 8 kv heads, RoPE, causal) Trainium2 Bass kernel.

Sharding: tensor-parallel over kv heads -- core c owns kv head c and q heads
4c..4c+3 for both batches. Each core computes a partial o-projection
(its 256 attn channels x Wo columns) in fp16 and the host sums the 8 partials.

Device-side structure (per core, per batch):
  * QKV projection in fp16: stationary x chunks [d,128 tok], moving fused
    W [d, 384] = [4 q heads | k | v]; psum evicted to fp16 by DVE.
  * RoPE on GPSIMD in token-partition layout (keeps DVE clear for the
    attention stream); head dims are host-permuted to [32 evens | 32 odds]
    so the pair slices are stride-1.
  * Q/K transposed to [dh, tok] via DMA-XBAR transposes (no PE involvement);
    K replicated to partitions 64:127 with one SBUF-to-SBUF DMA per batch so
    odd heads can use matching base partitions.
  * Scores computed [keys, q] per 128-key chunk with the moving operand
    clipped to the causal range (exact-causal FLOPs/exp); exp on ACT
    (scale=1/8, no max needed); diagonal triangle masked by DVE fp16 mul.
  * attn.V restructured: out [q, dh+1] psum (65 rows per 128-key chunk, half
    the PE cost of the [dh, q] form); ones column of V gives the softmax
    denominator; normalization = DVE reciprocal + per-partition scalar mul.
    Each of the 4 q-subchunk accumulators owns a full psum bank (hardware
    accumulation groups are per-bank and must not interleave within one).
  * Normalized attn tiles [q, dh] are pair-packed and DMA-XBAR-transposed
    into the o-projection stationary layout [channels, tok].
  * o-projection psum evicted to fp16 by DVE/ACT (GPSIMD cannot read PSUM);
    o written back fp16 by GPSIMD-issued SWDGE DMAs (off the shared HWDGE).
  * Cross-phase software pipelining: attention starts after 4 QKV tiles;
    remaining QKV(b0) tiles, QKV(b1), and o-proj(b0) PE work drain as
    background units between score pieces of the ACT-paced attention phases.
"""

import numpy as np
from collections import deque
from contextlib import ExitStack

import concourse.bass as bass
from concourse import bacc
import concourse.mybir as mybir
import concourse.tile as tile
from concourse.bass_utils import run_bass_kernel_spmd

B, S, D = 2, 2048, 2048
DH = 64            # head dim
G = 4              # q heads per core (= per kv head)
NCORES = 8
TT = 512           # attention i-tile
NTT = S // TT      # 4
KC = D // 128      # 16 contraction chunks
NJC = S // 128     # 16 token/key chunks of 128
F32 = mybir.dt.float32
F16 = mybir.dt.float16
ROPE_BASE = 10000.0

_cached = {}


def build_nc():
    nc = bacc.Bacc("TRN2", target_bir_lowering=False, debug=False)
    xt = nc.declare_dram_parameter("xt", [B, D, S], F16, isOutput=False)
    wall = nc.declare_dram_parameter("wall", [D, 384], F16, isOutput=False)
    wot = nc.declare_dram_parameter("wot", [256, D], F16, isOutput=False)
    cosr = nc.declare_dram_parameter("cosr", [S, 32], F16, isOutput=False)
    sinr = nc.declare_dram_parameter("sinr", [S, 32], F16, isOutput=False)
    cmask = nc.declare_dram_parameter("cmask", [4, 128, TT], F16, isOutput=False)
    o = nc.declare_dram_parameter("o", [B, S, D], F16, isOutput=True)

    EXP = mybir.ActivationFunctionType.Exp

    with tile.TileContext(nc) as tc, ExitStack() as ctx:
        wpool = ctx.enter_context(tc.tile_pool(name="weights", bufs=1))
        per_b = ctx.enter_context(tc.tile_pool(name="per_b", bufs=1))
        xpool = ctx.enter_context(tc.tile_pool(name="xstream", bufs=4))
        qkvpool = ctx.enter_context(tc.tile_pool(name="qkv", bufs=4))
        epool = ctx.enter_context(tc.tile_pool(name="exp", bufs=7))
        rpool = ctx.enter_context(tc.tile_pool(name="rope", bufs=3))
        opool = ctx.enter_context(tc.tile_pool(name="out", bufs=8))
        spool = ctx.enter_context(tc.tile_pool(name="small", bufs=4))
        # psum budget (8 banks): pgen 2 (shared by qkv-proj and o-proj),
        # scores 2, av accumulators 4.
        pp_gen = ctx.enter_context(tc.tile_pool(name="pgen", bufs=2, space="PSUM"))
        pp_att = ctx.enter_context(tc.tile_pool(name="patt", bufs=2, space="PSUM"))
        pp_av = ctx.enter_context(tc.tile_pool(name="pav", bufs=1, space="PSUM"))

        # ---- persistent weights/tables ----
        wall_sb = wpool.tile([128, KC, 384], F16, tag="wall")
        wot_sb = wpool.tile([128, 2, D], F16, tag="wot")
        cos_sb = wpool.tile([128, NJC, 32], F16, tag="cos")
        sin_sb = wpool.tile([128, NJC, 32], F16, tag="sin")
        mask_sb = wpool.tile([128, 1, 128], F16, tag="mask")

        def load_wall_head():
            nc.sync.dma_start(wall_sb[:, 0:4, :],
                              wall.rearrange("(kc p) n -> p kc n", p=128)[:, 0:4, :])

        def load_weights():
            nc.sync.dma_start(wall_sb[:, 4:KC, :],
                              wall.rearrange("(kc p) n -> p kc n", p=128)[:, 4:KC, :])
            nc.sync.dma_start(cos_sb[:],
                              cosr.rearrange("(j p) n -> p j n", p=128))
            nc.sync.dma_start(sin_sb[:],
                              sinr.rearrange("(j p) n -> p j n", p=128))

        def load_weights_late():
            # needed from the first attention diagonal onward
            nc.sync.dma_start(mask_sb[:, 0, 0:128], cmask[0][:, 0:128])
            # warm the Exp activation table off the critical path
            warm = spool.tile([128, 1], F32, tag="warm")
            nc.vector.memset(warm[:], 0.0)
            nc.scalar.activation(warm[:], warm[:], EXP, scale=1.0)

        def load_wot():
            # needed only by the o-projection, much later
            nc.sync.dma_start(wot_sb[:],
                              wot.rearrange("(cc p) n -> p cc n", p=128))

        # per-batch persistent tiles
        qt = {}     # [128, 2, S]: pair p holds heads 2p (part 0:64), 2p+1 (64:128)
        ktv = {}    # [128, S]: rows 0:64 = K^T; rows 64:128 = V^T (unused)
        ktv2 = {}   # [128, S]: rows 64:128 = K^T copy (for odd heads)
        vsb = {}    # [128, NJC, 65]: V natural [tok, dh] + ones column
        at = {}     # [128, 2, S]: o-proj stationary (channels x tokens)
        nrm = {}    # [128, 2, NJC, 128]: normalized attn [q, dh] pair-packed
        for b in range(B):
            qt[b] = per_b.tile([128, 2, S], F16, tag=f"qt{b}", name=f"qt{b}")
            ktv[b] = per_b.tile([128, S], F16, tag=f"ktv{b}", name=f"ktv{b}")
            ktv2[b] = per_b.tile([128, S], F16, tag=f"ktv2{b}", name=f"ktv2{b}")
            vsb[b] = per_b.tile([128, NJC, 65], F16, tag=f"vsb{b}", name=f"vsb{b}")
            at[b] = per_b.tile([128, 2, S], F16, tag=f"at{b}", name=f"at{b}")
            nrm[b] = per_b.tile([128, 2, NJC, 128], F16, tag=f"nrm{b}",
                                name=f"nrm{b}")
            nc.vector.memset(vsb[b][:, :, 64:65], 1.0)

        # ---------- QKV projection + rope (emitted as interleavable units) ----
        def qkv_units(b):
            """Closures, each emitting a quarter-tile of projection work
            (4 PE matmuls); the last quarter adds DVE evict/rope + SP
            transposes. Fine granularity lets attention interleave them."""
            units = []
            xts = {}
            pqs = {}

            def load_x(tg):
                def emit():
                    xts[tg] = xpool.tile([128, KC, TT], F16, tag="xt",
                                         name=f"xt_{b}_{tg}")
                    nc.sync.dma_start(
                        xts[tg][:],
                        xt[b].rearrange("(kc p) s -> p kc s", p=128)
                        [:, :, tg * TT:(tg + 1) * TT])
                return emit

            def make_unit(tt, quarter):
                def emit():
                    tg = tt // 4
                    xtile = xts[tg]
                    s0 = (tt % 4) * 128
                    if quarter == 0:
                        pqs[tt] = pp_gen.tile([128, TT], F32, tag="pg",
                                              name=f"pq_{b}_{tt}")
                    pq = pqs[tt]
                    for k in range(quarter * 4, quarter * 4 + 4):
                        nc.tensor.matmul(pq[:, 0:384], xtile[:, k, s0:s0 + 128],
                                         wall_sb[:, k, :],
                                         start=(k == 0), stop=(k == KC - 1))
                    if quarter < 3:
                        return
                    del pqs[tt]
                    qkv = qkvpool.tile([128, 384], F16, tag="qkv")
                    nc.vector.tensor_copy(qkv[:], pq[:, 0:384])
                    # rope on q+k (cols 0:320); host permuted each head's dims
                    # to [32 evens | 32 odds] so these slices are stride-1
                    pear = qkv[:, 0:320].rearrange("p (h half i) -> p h half i",
                                                   half=2, i=32)
                    ev, od = pear[:, :, 0, :], pear[:, :, 1, :]
                    # one 32-wide table broadcast (stride-0) across 5 heads
                    cs = cos_sb[:, tt:tt + 1, :].broadcast_to([128, 5, 32])
                    sn = sin_sb[:, tt:tt + 1, :].broadcast_to([128, 5, 32])
                    ec = rpool.tile([128, 5, 32], F16, tag="ec")
                    es = rpool.tile([128, 5, 32], F16, tag="es")
                    oc = rpool.tile([128, 5, 32], F16, tag="oc")
                    os_ = rpool.tile([128, 5, 32], F16, tag="os")
                    nc.gpsimd.tensor_mul(ec[:], ev, cs)
                    nc.gpsimd.tensor_mul(es[:], ev, sn)
                    nc.gpsimd.tensor_mul(oc[:], od, cs)
                    nc.gpsimd.tensor_mul(os_[:], od, sn)
                    nc.gpsimd.tensor_sub(ev, ec[:], os_[:])
                    nc.gpsimd.tensor_add(od, es[:], oc[:])
                    nc.gpsimd.tensor_copy(vsb[b][:, tt, 0:64], qkv[:, 320:384])
                    tsl = slice(tt * 128, (tt + 1) * 128)
                    nc.sync.dma_start_transpose(qt[b][:, 0, tsl], qkv[:, 0:128])
                    nc.sync.dma_start_transpose(qt[b][:, 1, tsl], qkv[:, 128:256])
                    nc.sync.dma_start_transpose(ktv[b][:, tsl], qkv[:, 256:384])
                return emit

            for tt in range(NJC):
                if tt % 4 == 0:
                    units.append(load_x(tt // 4))
                for quarter in range(4):
                    units.append(make_unit(tt, quarter))
            return units

        def emit_ktv2(b):
            # K^T replicated to partitions 64:127 (DMA moves across partitions)
            nc.sync.dma_start(ktv2[b][64:128, :], ktv[b][0:64, :])

        # ---------- attention (q-outer, exact-causal via clipped moving) ----
        # Per (g, 512-query tile): key chunks jc ascending; the moving operand
        # is clipped to the causal range q >= 128*jc, so score/exp work is the
        # exact causal set. The 4 q-subchunk accumulators [q, dh+1] each own a
        # full psum bank -- hardware accumulation groups are per-bank, so
        # groups may interleave across banks but never within one.
        def emit_attn(b, bg, front=0):
            """bg: deque of PE-work closures interleaved between score pieces.
            The first `front` units drain at 4/piece (they gate upcoming
            attention tiles); the rest spread uniformly."""
            n_pieces = G * sum(4 * it + 4 for it in range(NTT))
            quota = [0] * n_pieces
            nfront = min(front, len(bg))
            pf = (nfront + 3) // 4
            for i in range(pf):
                quota[i] = min(4, nfront - 4 * i)
            rest = len(bg) - nfront
            if rest > 0:
                for i, extra in enumerate(np.diff(np.round(
                        np.linspace(0, rest, n_pieces - pf + 1)).astype(int))):
                    quota[pf + i] += int(extra)
            piece_idx = 0
            for g in range(G):
                base, pair, cc = (g % 2) * 64, g // 2, g // 2
                kst = ktv[b] if g % 2 == 0 else ktv2[b]
                for it in range(NTT):
                    avs = [pp_av.tile([128, TT], F32, tag=f"av{sub}",
                                      name=f"av_{b}_{g}_{it}_{sub}")
                           for sub in range(4)]
                    njc = 4 * it + 4
                    pending = []

                    def norm_one(qc, sub, avs=avs):
                        rcp = spool.tile([128, 1], F32, tag="rcp")
                        nc.vector.reciprocal(rcp[:], avs[sub][:, 64:65])
                        nc.vector.tensor_scalar_mul(
                            nrm[b][:, cc, qc, base:base + 64],
                            avs[sub][:, 0:64], rcp[:])

                    def flush_one(pending=pending, avs=avs, it=it):
                        jd, q0d, wd, ed = pending.pop(0)
                        for i in range(wd // 128):
                            qc = q0d // 128 + i
                            sub = qc - 4 * it
                            nc.tensor.matmul(
                                avs[sub][:, 0:65],
                                ed[:, i * 128:(i + 1) * 128],
                                vsb[b][:, jd, :],
                                start=(jd == 0), stop=(jd == qc))
                            if jd == qc:
                                norm_one(qc, sub)

                    for jc in range(njc):
                        jsl = slice(jc * 128, (jc + 1) * 128)
                        q0 = max(jc * 128, it * TT)
                        w = (it + 1) * TT - q0
                        psc = pp_att.tile([128, TT], F32, tag="sc")
                        nc.tensor.matmul(
                            psc[:, 0:w], kst[base:base + 64, jsl],
                            qt[b][base:base + 64, pair, q0:q0 + w],
                            start=True, stop=True)
                        esb = epool.tile([128, TT], F16, tag="exp")
                        nc.scalar.activation(esb[:, 0:w], psc[:, 0:w], EXP,
                                             scale=0.125)
                        if q0 == jc * 128:  # diagonal block: triangular mask
                            nc.vector.tensor_mul(esb[:, 0:128], esb[:, 0:128],
                                                 mask_sb[:, 0, 0:128])
                        pending.append(((0, jc, q0, w),))
                        pending[-1] = (jc, q0, w, esb)
                        if len(pending) > 4:
                            flush_one()
                        for _ in range(quota[piece_idx]):
                            if bg:
                                bg.popleft()()
                        piece_idx += 1
                    while pending:
                        flush_one()
                if g % 2 == 1:  # pair done: transpose into o-proj layout
                    for qc in range(NJC):
                        nc.sync.dma_start_transpose(
                            at[b][:, cc, qc * 128:(qc + 1) * 128],
                            nrm[b][:, cc, qc, :])
            while bg:
                bg.popleft()()

        # ---------- o projection (partial over this core's 256 channels) ----
        def oproj_units(b, evict_engines, pools):
            """evict_engines / pools: rotations for the psum->SBUF eviction
            engine and the psum pool (spread so neither serializes)."""
            units = []
            obs = {}

            def make_unit(tt, nt):
                def emit():
                    tsl = slice(tt * 128, (tt + 1) * 128)
                    if nt == 0:
                        obs[tt] = opool.tile([128, 4, TT], F16, tag="osb",
                                             name=f"osb_{b}_{tt}")
                    ob = obs[tt]
                    nsl = slice(nt * TT, (nt + 1) * TT)
                    pool = pools[(tt * 4 + nt) % len(pools)]
                    po = pool.tile([128, TT], F32,
                                   tag="pg" if pool is pp_gen else "sc",
                                   name=f"po_{b}_{tt}_{nt}")
                    nc.tensor.matmul(po[:], at[b][:, 0, tsl],
                                     wot_sb[:, 0, nsl], start=True, stop=False)
                    nc.tensor.matmul(po[:], at[b][:, 1, tsl],
                                     wot_sb[:, 1, nsl], start=False, stop=True)
                    eng = evict_engines[(tt * 4 + nt) % len(evict_engines)]
                    if eng is nc.scalar:
                        eng.copy(ob[:, nt, :], po[:])
                    else:
                        eng.tensor_copy(ob[:, nt, :], po[:])
                    if nt == 3:
                        del obs[tt]
                        nc.gpsimd.dma_start(o[b, tsl, :], ob[:])
                return emit

            for tt in range(NJC):
                for nt in range(4):
                    units.append(make_unit(tt, nt))
            return units

        # ---------- schedule ----------
        u0 = qkv_units(0)
        load_wall_head()  # first 4 contraction chunks of W: unblocks matmuls
        u0[0]()           # first x tile load
        load_weights()
        for u in u0[1:17]:  # tiles 0-3: enough to start attention it-tile 0
            u()
        load_weights_late()
        # remaining qkv(0) tiles drain front-loaded into attn(0) (they gate
        # its later it-tiles), then qkv(1) spreads across the rest
        u1 = qkv_units(1)
        ncut = 40  # last 4 tiles of qkv(1): only needed by attn(1) it-tile 3
        bg0 = deque(list(u0[17:]) + [lambda: emit_ktv2(0), load_wot]
                    + u1[:-ncut])
        emit_attn(0, bg0, front=len(u0) - 17 + 2)
        # during attn(1): evict on DVE (GPSIMD cannot read PSUM; ACT is
        # exp-bound) and stay off the attention psum pools
        bg1 = deque(u1[-ncut:] + [lambda: emit_ktv2(1)]
                    + oproj_units(0, [nc.vector], [pp_gen]))
        emit_attn(1, bg1, front=ncut + 1)
        # tail: every non-PE engine and the score psum pool are idle
        for u in oproj_units(1, [nc.vector, nc.scalar], [pp_gen, pp_att]):
            u()
    nc.compile()
    return nc


def host_inputs(x, Wq, Wk, Wv, Wo):
    """Per-core input maps. Q/K weight rows permuted per head to
    [32 even dims | 32 odd dims] so device rope slices are stride-1."""
    xtp = np.ascontiguousarray(
        np.transpose(np.asarray(x, np.float32), (0, 2, 1))).astype(np.float16)
    inv = ROPE_BASE ** (-np.arange(0, DH, 2, dtype=np.float64) / DH)
    th = np.arange(S, dtype=np.float64)[:, None] * inv[None, :]  # (S, 32)
    cosr = np.cos(th).astype(np.float16)  # (S, 32)
    sinr = np.sin(th).astype(np.float16)
    p = np.arange(128)[:, None]
    f = np.arange(TT)[None, :]
    cmask = np.stack([(p + m * 128 <= f).astype(np.float16) for m in range(4)])
    perm = np.concatenate([np.arange(0, DH, 2), np.arange(1, DH, 2)])
    in_maps = []
    for c in range(NCORES):
        qrows = [Wq[(4 * c + h) * DH:(4 * c + h + 1) * DH][perm] for h in range(G)]
        krows = Wk[DH * c:DH * (c + 1)][perm]
        vrows = Wv[DH * c:DH * (c + 1)]
        wall = np.concatenate(qrows + [krows, vrows], axis=0)     # (384, D)
        wall = np.ascontiguousarray(wall.T.astype(np.float16))    # (D, 384)
        wot = np.ascontiguousarray(Wo[:, 256 * c:256 * (c + 1)].T
                                   .astype(np.float16))           # (256, D)
        in_maps.append(dict(xt=xtp, wall=wall, wot=wot, cosr=cosr, sinr=sinr,
                            cmask=cmask))
    return in_maps


def kernel(**inputs):
    x = np.asarray(inputs["x"], dtype=np.float32)
    Wq = np.asarray(inputs["Wq"], dtype=np.float32)
    Wk = np.asarray(inputs["Wk"], dtype=np.float32)
    Wv = np.asarray(inputs["Wv"], dtype=np.float32)
    Wo = np.asarray(inputs["Wo"], dtype=np.float32)
    in_maps = host_inputs(x, Wq, Wk, Wv, Wo)
    if "nc" not in _cached:
        _cached["nc"] = build_nc()
    res = run_bass_kernel_spmd(_cached["nc"], in_maps, list(range(NCORES)))
    out = np.zeros((B, S, D), np.float64)
    for r in res.results:
        out += np.asarray(r["o"], np.float64)
    return out.astype(np.float32)
